# revision 2
# baseline (speedup 1.0000x reference)
"""Block-local attention + LayerNorm for Trainium2, v6: bf16 scores +
fp8-DoubleRow PV path.

HW findings driving this version:
  - bf16 matmuls (N=512) sustain ~112ns only when consecutive matmuls target
    DIFFERENT psum banks; 4-matmul accumulation runs into one bank serialize
    fill with drain.  All big matmul loops interleave two accumulation
    groups so consecutive PE instructions alternate banks.
  - A and V phases are interleaved so the ACT at-copies are done before the
    S phase needs them; the rowsum matmuls are emitted after the previous
    block's O phase so the exp/mask chain has time to finish.

Math per block (see reference.py):
  S^T = X (X W12)^T, W12 = W1 W2^T/sqrt(d); P_u^T = exp(S^T) * mask^T
  V = X W3; O_u = P_u V (lhsT = P_u^T directly); rowsum via N=1 matmuls
  y = LN(O_u + diag(rowsum) X)   [LN scale-invariance; eps * rowsum^2]
"""

import math
import sys

import numpy as np
import ml_dtypes

sys.path.insert(0, "/opt/trn_rl_repo")

import concourse.bacc as bacc
import concourse.tile as tile
from concourse import masks, mybir
from concourse.bass_utils import run_bass_kernel_spmd

DIM = 512
BLOCK_NUM = 16
BLOCK_LEN = 512
BATCH = 4
LN_EPS = 1e-3
N_CORES = 8
NBLK = (BATCH * BLOCK_NUM) // N_CORES
P = 128
NCH = DIM // P

F32 = mybir.dt.float32
BF16 = mybir.dt.bfloat16
E4 = mybir.dt.float8e4
E5 = mybir.dt.float8e5
I32 = mybir.dt.int32
DR = mybir.MatmulPerfMode.DoubleRow

LN4 = math.log(4.0)
SW3 = 16.0
MASKVAL = 28672.0  # e5m2-exact; exp(s - 28672) == 0 in fp32

BFnp = ml_dtypes.bfloat16
E4np = ml_dtypes.float8_e4m3
E5np = ml_dtypes.float8_e5m2


def build_nc(nblk=NBLK, repeat=1):
    nc = bacc.Bacc("TRN2", target_bir_lowering=False, debug=False,
                   num_devices=N_CORES)

    xtb_d = nc.declare_dram_parameter("xtb", [nblk, P, NCH, DIM], BF16, isOutput=False)
    xt8_d = nc.declare_dram_parameter("xt8", [nblk, P, NCH, DIM], E4, isOutput=False)
    mkt_d = nc.declare_dram_parameter("mkt", [nblk, P, NCH, DIM], E5, isOutput=False)
    xn_d = nc.declare_dram_parameter("xn", [nblk, P, NCH, DIM], BF16, isOutput=False)
    w12_d = nc.declare_dram_parameter("w12", [P, NCH, DIM], BF16, isOutput=False)
    w38_d = nc.declare_dram_parameter("w38", [P, NCH, DIM], E4, isOutput=False)
    out_d = nc.declare_dram_parameter("out", [nblk, P, NCH, DIM], BF16, isOutput=True)

    with tile.TileContext(nc) as tc:
        with (
            tc.tile_pool(name="const", bufs=1) as const,
            tc.tile_pool(name="xtb", bufs=3) as p_xtb,
            tc.tile_pool(name="xt8", bufs=3) as p_xt8,
            tc.tile_pool(name="mkt", bufs=2) as p_mkt,
            tc.tile_pool(name="xn", bufs=2) as p_xn,
            tc.tile_pool(name="at", bufs=2) as p_at,
            tc.tile_pool(name="v", bufs=2) as p_v,
            tc.tile_pool(name="put", bufs=2) as p_put,
            tc.tile_pool(name="rs", bufs=2) as p_rs,
            tc.tile_pool(name="ob", bufs=2) as p_ob,
            tc.tile_pool(name="tiny", bufs=4) as p_tiny,
            tc.tile_pool(name="ps_mm", bufs=4, space="PSUM") as ps_mm,
            tc.tile_pool(name="ps_o", bufs=4, space="PSUM") as ps_o,
        ):
            w12_sb = const.tile([P, NCH, DIM], BF16)
            nc.sync.dma_start(out=w12_sb[:, 0:2, :], in_=w12_d[:, 0:2, :])
            nc.sync.dma_start(out=w12_sb[:, 2:4, :], in_=w12_d[:, 2:4, :])
            w38_sb = const.tile([P, NCH, DIM], E4)
            nc.sync.dma_start(out=w38_sb, in_=w38_d[:])
            ones8 = const.tile([P, 1], E4)
            nc.vector.memset(ones8, 1.0)
            bias_ln4 = const.tile([P, 1], F32)
            nc.vector.memset(bias_ln4, -LN4)

            def stage1(b):
                st = {"b": b}
                xtb = p_xtb.tile([P, NCH, DIM], BF16, tag="xtb")
                nc.sync.dma_start(out=xtb[:, 0:2, :], in_=xtb_d[b, :, 0:2, :])
                nc.sync.dma_start(out=xtb[:, 2:4, :], in_=xtb_d[b, :, 2:4, :])
                xt8 = p_xt8.tile([P, NCH, DIM], E4, tag="xt8")
                nc.sync.dma_start(out=xt8, in_=xt8_d[b])
                mkt = p_mkt.tile([P, NCH, DIM], E5, tag="mkt")
                nc.sync.dma_start(out=mkt, in_=mkt_d[b])
                xn = p_xn.tile([P, NCH, DIM], BF16, tag="xn")
                nc.sync.dma_start(out=xn, in_=xn_d[b])

                at = p_at.tile([P, NCH, DIM], BF16, tag="at")
                v = p_v.tile([P, NCH, DIM], E4, tag="v")

                # A^T and V, interleaved in pairs: consecutive matmuls
                # alternate psum banks, and at-copies land well before S
                for half in range(2):
                    c0, c1 = 2 * half, 2 * half + 1
                    psa = ps_mm.tile([P, DIM], F32, tag="mm")
                    psb = ps_mm.tile([P, DIM], F32, tag="mm")
                    for dc in range(NCH):
                        nc.tensor.matmul(
                            psa[:],
                            lhsT=w12_sb[:, dc, c0 * P:(c0 + 1) * P],
                            rhs=xtb[:, dc, :],
                            start=(dc == 0), stop=(dc == NCH - 1))
                        nc.tensor.matmul(
                            psb[:],
                            lhsT=w12_sb[:, dc, c1 * P:(c1 + 1) * P],
                            rhs=xtb[:, dc, :],
                            start=(dc == 0), stop=(dc == NCH - 1))
                    nc.scalar.copy(at[:, c0, :], psa[:])
                    nc.scalar.copy(at[:, c1, :], psb[:])
                    psa = ps_mm.tile([P, DIM], F32, tag="mm")
                    psb = ps_mm.tile([P, DIM], F32, tag="mm")
                    for j in range(2):
                        nc.tensor.matmul(
                            psa[:],
                            lhsT=xt8[:, 2 * j:2 * j + 2, c0 * P:(c0 + 1) * P],
                            rhs=w38_sb[:, 2 * j:2 * j + 2, :],
                            start=(j == 0), stop=(j == 1), perf_mode=DR)
                        nc.tensor.matmul(
                            psb[:],
                            lhsT=xt8[:, 2 * j:2 * j + 2, c1 * P:(c1 + 1) * P],
                            rhs=w38_sb[:, 2 * j:2 * j + 2, :],
                            start=(j == 0), stop=(j == 1), perf_mode=DR)
                    nc.scalar.mul(v[:, c0, :], psa[:], 1.0 / SW3)
                    nc.scalar.mul(v[:, c1, :], psb[:], 1.0 / SW3)

                # S^T in interleaved pairs -> exp -> *mask
                put = p_put.tile([P, NCH, DIM], E4, tag="put")
                for half in range(2):
                    k0, k1 = 2 * half, 2 * half + 1
                    psa = ps_mm.tile([P, DIM], F32, tag="mm")
                    psb = ps_mm.tile([P, DIM], F32, tag="mm")
                    for d2c in range(NCH):
                        nc.tensor.matmul(
                            psa[:],
                            lhsT=xtb[:, d2c, k0 * P:(k0 + 1) * P],
                            rhs=at[:, d2c, :],
                            start=(d2c == 0), stop=(d2c == NCH - 1))
                        nc.tensor.matmul(
                            psb[:],
                            lhsT=xtb[:, d2c, k1 * P:(k1 + 1) * P],
                            rhs=at[:, d2c, :],
                            start=(d2c == 0), stop=(d2c == NCH - 1))
                    for kc, ps in ((k0, psa), (k1, psb)):
                        nc.vector.tensor_add(ps[:], ps[:], mkt[:, kc, :])
                        nc.scalar.activation(put[:, kc, :], ps[:],
                                             mybir.ActivationFunctionType.Exp,
                                             bias=bias_ln4[:])
                st.update(xn=xn, put=put, v=v)
                return st

            def rowsum(st):
                # rowsum[q] = sum_k P_u[q,k] in [128, 4] orientation directly:
                # lhsT = P_u^T chunk (stationary), rhs = ones column (N=1)
                put = st["put"]
                psr = ps_mm.tile([P, NCH], F32, tag="mm")
                for qc in range(NCH):
                    for kc in range(NCH):
                        nc.tensor.matmul(
                            psr[:, qc:qc + 1],
                            lhsT=put[:, kc, qc * P:(qc + 1) * P],
                            rhs=ones8[:],
                            start=(kc == 0), stop=(kc == NCH - 1))
                rs_col = p_rs.tile([P, NCH], F32, tag="rscol")
                nc.vector.tensor_copy(rs_col[:], psr[:])
                st["rs_col"] = rs_col

            def stage2(st):
                xn, put, v, rs_col = st["xn"], st["put"], st["v"], st["rs_col"]

                # O_u = P_u V (fp8 DR); residual rowsum*X added on DVE
                mvb = p_tiny.tile([P, NCH, 2], F32, tag="mvb")
                psos = [None] * NCH
                for half in range(2):
                    q0, q1 = 2 * half, 2 * half + 1
                    psos[q0] = ps_o.tile([P, DIM], F32, tag="o", name=f"pso{q0}")
                    psos[q1] = ps_o.tile([P, DIM], F32, tag="o", name=f"pso{q1}")
                    for j in range(2):
                        for q in (q0, q1):
                            nc.tensor.matmul(
                                psos[q][:],
                                lhsT=put[:, 2 * j:2 * j + 2, q * P:(q + 1) * P],
                                rhs=v[:, 2 * j:2 * j + 2, :],
                                start=(j == 0), stop=(j == 1), perf_mode=DR)
                    for q in (q0, q1):
                        # O_pre = O_u + rowsum*X  (in-place on psum)
                        nc.vector.scalar_tensor_tensor(
                            out=psos[q][:], in0=xn[:, q, :],
                            scalar=rs_col[:, q:q + 1], in1=psos[q][:],
                            op0=mybir.AluOpType.mult, op1=mybir.AluOpType.add)
                        stats = p_tiny.tile([P, 6], F32, tag="stats")
                        nc.vector.bn_stats(stats[:], psos[q][:])
                        nc.vector.bn_aggr(mvb[:, q, :], stats[:])

                # LN tail: istd = rsqrt(var + eps*rowsum^2), magic + 2 Newton
                rs2 = p_tiny.tile([P, NCH], F32, tag="rs2")
                nc.vector.tensor_mul(rs2[:], rs_col[:], rs_col[:])
                tv = p_tiny.tile([P, NCH], F32, tag="tv")
                nc.vector.scalar_tensor_tensor(
                    out=tv[:], in0=rs2[:], scalar=LN_EPS, in1=mvb[:, :, 1],
                    op0=mybir.AluOpType.mult, op1=mybir.AluOpType.add)
                yv = p_tiny.tile([P, NCH], F32, tag="yv")
                hv = p_tiny.tile([P, NCH], F32, tag="hv")
                nc.vector.tensor_scalar(
                    out=hv[:].bitcast(I32), in0=tv[:].bitcast(I32),
                    scalar1=1, scalar2=None,
                    op0=mybir.AluOpType.logical_shift_right)
                nc.vector.tensor_scalar(
                    out=yv[:].bitcast(I32), in0=hv[:].bitcast(I32),
                    scalar1=-1, scalar2=0x5F3759DF,
                    op0=mybir.AluOpType.mult, op1=mybir.AluOpType.add)
                av = p_tiny.tile([P, NCH], F32, tag="av")
                cv = p_tiny.tile([P, NCH], F32, tag="cv")
                for _ in range(2):
                    nc.vector.tensor_mul(av[:], yv[:], yv[:])
                    nc.vector.tensor_mul(av[:], av[:], tv[:])
                    nc.vector.tensor_scalar(
                        out=cv[:], in0=av[:], scalar1=-0.5, scalar2=1.5,
                        op0=mybir.AluOpType.mult, op1=mybir.AluOpType.add)
                    nc.vector.tensor_mul(yv[:], yv[:], cv[:])
                negms = p_tiny.tile([P, NCH], F32, tag="negms")
                nc.vector.tensor_mul(negms[:], mvb[:, :, 0], yv[:])
                nc.vector.tensor_scalar_mul(negms[:], negms[:], -1.0)

                ob = p_ob.tile([P, NCH, DIM], BF16, tag="ob")
                for qc in range(2):
                    nc.vector.tensor_scalar(
                        out=ob[:, qc, :], in0=psos[qc][:],
                        scalar1=yv[:, qc:qc + 1], scalar2=negms[:, qc:qc + 1],
                        op0=mybir.AluOpType.mult, op1=mybir.AluOpType.add)
                for qc in range(2, NCH):
                    nc.scalar.activation(
                        ob[:, qc, :], psos[qc][:],
                        mybir.ActivationFunctionType.Identity,
                        bias=negms[:, qc:qc + 1], scale=yv[:, qc:qc + 1])
                nc.scalar.dma_start(out=out_d[st["b"]], in_=ob[:])

            def _blocks():
                prev = None
                for b in range(nblk):
                    st = stage1(b)
                    if prev is not None:
                        stage2(prev)
                    rowsum(st)
                    prev = st
                stage2(prev)

            if repeat == 1:
                _blocks()
            else:
                with tc.For_i(0, repeat, 1):
                    _blocks()

    nc.finalize()
    return nc


_NC_CACHE = {}


def _get_nc():
    if "nc" not in _NC_CACHE:
        _NC_CACHE["nc"] = build_nc()
    return _NC_CACHE["nc"]


def prep_in_maps(inputs, mask_array, dw1, dw2, dw3, db1, db2, db3):
    nb = BATCH * BLOCK_NUM
    x = np.asarray(inputs, np.float32).reshape(nb, BLOCK_LEN, DIM)
    m = np.asarray(mask_array, np.float32).reshape(nb, BLOCK_LEN, DIM)

    xt = np.ascontiguousarray(
        x.reshape(nb, BLOCK_LEN, NCH, P).transpose(0, 3, 2, 1))
    xtb = xt.astype(BFnp)
    xt8 = xt.astype(E4np)
    # additive mask bias, e5m2-exact: 0 where m==1, -28672 where m==0
    mkt = np.ascontiguousarray(
        ((m - 1.0) * np.float32(MASKVAL))
        .reshape(nb, BLOCK_LEN, NCH, P).transpose(0, 3, 2, 1)).astype(E5np)
    xn_nat = x.reshape(nb, NCH, P, DIM).transpose(0, 2, 1, 3)
    db3 = np.asarray(db3, np.float32)
    if db3.any():
        xn_nat = xn_nat + db3[None, None, None, :]
    xn = np.ascontiguousarray(xn_nat).astype(BFnp)

    scale = np.float32(1.0 / math.sqrt(DIM))
    w12 = ((np.asarray(dw1, np.float32) @ np.asarray(dw2, np.float32).T) * scale)
    w12 = np.ascontiguousarray(
        w12.reshape(NCH, P, DIM).transpose(1, 0, 2)).astype(BFnp)
    w38 = np.ascontiguousarray(
        (np.asarray(dw3, np.float32) * np.float32(SW3))
        .reshape(NCH, P, DIM).transpose(1, 0, 2)).astype(E4np)

    in_maps = []
    for c in range(N_CORES):
        s = slice(c * NBLK, (c + 1) * NBLK)
        in_maps.append({"xtb": xtb[s], "xt8": xt8[s], "mkt": mkt[s],
                        "xn": xn[s], "w12": w12, "w38": w38})
    return in_maps


def kernel(inputs, mask_array, dw1, dw2, dw3, db1, db2, db3):
    nc = _get_nc()
    in_maps = prep_in_maps(inputs, mask_array, dw1, dw2, dw3, db1, db2, db3)
    res = run_bass_kernel_spmd(nc, in_maps, list(range(N_CORES)))
    out = np.concatenate([res.results[c]["out"] for c in range(N_CORES)], axis=0)
    out = out.astype(np.float32)
    out = out.transpose(0, 2, 1, 3).reshape(BATCH, BLOCK_NUM, BLOCK_LEN, DIM)
    return np.ascontiguousarray(out)
